# revision 21
# baseline (speedup 1.0000x reference)
"""Diag-scale kernel: out = input * W (input @ diag(W)).

input: (16384, 4096) f32, W: (4096,) f32. Data-parallel over 8 NeuronCores:
each core gets 2048 rows; W (16KB) is sent to every core and replicated
across SBUF partitions on-chip. Memory-bound: each core streams 32 MiB in
and 32 MiB out, multiplying by W on the DVE in between.
"""

import os
import numpy as np

import concourse.bacc as bacc
import concourse.mybir as mybir
from concourse.tile import TileContext
from concourse.bass_utils import run_bass_kernel_spmd

N = 16384
D = 4096
NCORES = 8
ROWS = N // NCORES          # 2048 rows per core
P = 128                     # SBUF partitions
IO_BUFS = 5                 # 5 x 32KB/partition slots + 32KB W = 192KB cap

last_exec_time_ns = None
last_trace_dir = None
_built_nc = None


def _build():
    nc = bacc.Bacc(None, target_bir_lowering=False, debug=False)
    inp = nc.declare_dram_parameter("input", [ROWS, D], mybir.dt.float32, isOutput=False)
    w = nc.declare_dram_parameter("w", [1, D], mybir.dt.float32, isOutput=False)
    out = nc.declare_dram_parameter("out", [ROWS, D], mybir.dt.float32, isOutput=True)

    # chunk = (row_start, rows_per_partition, col_start, ncols).
    # Row-contiguous 4 MiB chunks (32KB per-partition descriptors are the
    # DMA-efficiency sweet spot); last chunk split 2+2 MiB to halve the
    # end-of-stream drain (last mul + last store).
    chunks = [(256 * k, 2, 0, D) for k in range(7)]
    chunks += [(1792, 1, 0, D), (1920, 1, 0, D)]

    with TileContext(nc) as tc:
        with (
            tc.tile_pool(name="wpool", bufs=1) as wpool,
            tc.tile_pool(name="io", bufs=IO_BUFS) as io,
        ):
            wrow = wpool.tile([1, D], mybir.dt.float32)
            wt = wpool.tile([P, D], mybir.dt.float32)
            # 16KB W load + on-chip partition broadcast keeps W out of the
            # bulk-DMA budget entirely.
            nc.gpsimd.dma_start(out=wrow[:], in_=w[:, :])
            nc.gpsimd.partition_broadcast(wt[:], wrow[:], channels=P)
            for idx, (rs, r, c0, ncols) in enumerate(chunks):
                t = io.tile([P, r * ncols], mybir.dt.float32)
                # Loads alternate between the two HWDGE rings (SP and ACT)
                # so every SDMA engine always has load work from two
                # independent FIFOs; stores go via SWDGE (gpsimd) so their
                # completions land on the DMASW semaphore lanes — muls then
                # never falsely wait on stores through a shared round-robin
                # DMAHW lane, which otherwise stalls the drain phase.
                ldeng = nc.sync if idx % 2 == 0 else nc.scalar
                if r == 1:
                    src = inp[rs : rs + P, c0 : c0 + ncols]
                    dst = out[rs : rs + P, c0 : c0 + ncols]
                    ldeng.dma_start(out=t[:], in_=src)
                    nc.vector.tensor_mul(
                        out=t[:], in0=t[:], in1=wt[:, c0 : c0 + ncols]
                    )
                    nc.gpsimd.dma_start(out=dst, in_=t[:])
                else:
                    src = inp[rs : rs + P * r, :].rearrange("(p r) d -> p (r d)", r=r)
                    dst = out[rs : rs + P * r, :].rearrange("(p r) d -> p (r d)", r=r)
                    ldeng.dma_start(out=t[:], in_=src)
                    t3 = t[:].rearrange("p (r d) -> p r d", r=r)
                    nc.vector.tensor_mul(
                        out=t3, in0=t3, in1=wt[:, None, :].broadcast_to([P, r, D])
                    )
                    nc.gpsimd.dma_start(out=dst, in_=t[:])
    nc.compile()
    return nc


def kernel(input, W):
    global last_exec_time_ns, _built_nc
    input = np.ascontiguousarray(np.asarray(input, dtype=np.float32))
    W = np.asarray(W, dtype=np.float32).reshape(D)

    if _built_nc is None:
        _built_nc = _build()
    nc = _built_nc

    # single W row per core; replication across partitions happens on-chip
    w_rep = np.ascontiguousarray(W.reshape(1, D))
    shards = input.reshape(NCORES, ROWS, D)
    in_maps = [{"input": shards[c], "w": w_rep} for c in range(NCORES)]

    global last_trace_dir
    trace = os.environ.get("KERNEL_TRACE", "0") == "1"
    kwargs = {}
    if trace:
        import tempfile

        last_trace_dir = tempfile.mkdtemp(prefix="diag_trace_")
        kwargs = {"trace": True, "tmpdir": last_trace_dir}
    res = run_bass_kernel_spmd(nc, in_maps, core_ids=list(range(NCORES)), **kwargs)
    last_exec_time_ns = res.exec_time_ns

    out = np.concatenate([res.results[c]["out"] for c in range(NCORES)], axis=0)
    return out
